# revision 3
# baseline (speedup 1.0000x reference)
"""Multi-head causal attention (B=4, S=2048, D=1024, H=16) on 8 TRN2 cores.

Sharding: core c -> batch c//2, head-group c%2 (8 heads, 512 of the 1024
QKV columns / Wo rows).  Each core runs a fused QKV->attention->out-proj
kernel on its shard; the host sums the two head-group partials per batch.

Per-core layout choices:
  - x is fed pre-transposed (xT [D, S]) so Q^T/K^T come out of the PE in
    [m, s] layout and V in natural [s, m] layout with no on-chip transposes.
  - scores are computed transposed (S^T [k, q]); softmax runs as
    exp (ScalarE, scale=1/8 fused) -> causal mask (gpsimd affine_select,
    fill=0, diagonal tiles only, masked q-ranges skipped entirely) ->
    attnV matmul with a ones-column appended to V (M=65) so the softmax
    denominator accumulates for free in PSUM row 64.
  - normalization: DVE reciprocal of row 64, gpsimd partition_broadcast,
    one DVE multiply into C^T [m, s].
  - out-proj emits out^T [n, s]; the host transposes back.
All matmuls run as float32r (full PE rate at N>=256).
"""

import numpy as np

B, S, D = 4, 2048, 1024
H, DH = 16, 64
HPC = 8            # heads per core
M = HPC * DH       # 512: per-core qkv out dim / wo in dim
NCORE = 8
CH = 512           # q/s chunk size
NCH = S // CH      # 4
ND = D // 128      # 8  d-tiles (contraction for qkv proj)
NMT = M // 128     # 4  m-tiles (= head pairs)
NKT = S // 128     # 16 k-tiles
NNT = D // 128     # 8  n-tiles (out proj)

LAST_RESULT = None  # BassKernelResults of the most recent run (for test.py)


def _emit(nc, tc, tile, mybir, aps):
    import concourse.bass as bass  # noqa: F401

    f32 = mybir.dt.float32
    f32r = mybir.dt.float32r
    EXP = mybir.ActivationFunctionType.Exp
    xT, wq, wk, wv, wo, ones8, outT = aps

    def r(ap):
        return ap

    with (
        tc.tile_pool(name="w", bufs=1) as pw,
        tc.tile_pool(name="kv", bufs=1) as pkv,
        tc.tile_pool(name="qt", bufs=2) as pq,
        tc.tile_pool(name="ct", bufs=1) as pct,
        tc.tile_pool(name="x", bufs=1) as px,
        tc.tile_pool(name="u", bufs=4) as pu,
        tc.tile_pool(name="sm", bufs=2) as psm,
        tc.tile_pool(name="o", bufs=2) as po,
        tc.tile_pool(name="ps_proj", bufs=2, space="PSUM") as pp_proj,
        tc.tile_pool(name="ps_sc", bufs=2, space="PSUM") as pp_sc,
        tc.tile_pool(name="ps_av", bufs=2, space="PSUM") as pp_av,
        tc.tile_pool(name="ps_out", bufs=2, space="PSUM") as pp_out,
    ):
        # ---- weights ----
        wq_sb, wk_sb, wv_sb = [], [], []
        for d in range(ND):
            for lst, src, nm in ((wq_sb, wq, "wq"), (wk_sb, wk, "wk"), (wv_sb, wv, "wv")):
                t = pw.tile([128, M], f32r, name=f"{nm}{d}", tag=f"{nm}{d}")
                nc.sync.dma_start(
                    out=t, in_=src[128 * d:128 * (d + 1), :].bitcast(f32r)
                )
                lst.append(t)
        wo_sb = []
        for t in range(NMT):
            w = pw.tile([128, D], f32r, name=f"wo{t}", tag=f"wo{t}")
            nc.sync.dma_start(
                out=w, in_=wo[128 * t:128 * (t + 1), :].bitcast(f32r)
            )
            wo_sb.append(w)

        # ---- V storage: [s, 8 heads x (64 V + 1 ones)] ----
        vau = []
        for st in range(NKT):
            v = pkv.tile([128, HPC * 65], f32r, name=f"vau{st}", tag=f"vau{st}")
            nc.sync.dma_start(
                out=v.rearrange("p (h c) -> p h c", c=65)[:, :, 64:65],
                in_=ones8.rearrange("p (h c) -> p h c", c=1).bitcast(f32r),
            )
            vau.append(v)
        kt_sb = [[None] * NCH for _ in range(NMT)]

        for j in range(NCH):  # ---- chunk loop ----
            # x^T chunk [d, s]
            xt = []
            for d in range(ND):
                x_t = px.tile([128, CH], f32r, name=f"x{d}", tag=f"x{d}")
                nc.sync.dma_start(
                    out=x_t,
                    in_=xT[128 * d:128 * (d + 1), CH * j:CH * (j + 1)].bitcast(f32r),
                )
                xt.append(x_t)

            # Q^T, K^T projections -> [m, s]
            qt = []
            for t in range(NMT):
                ps = pp_proj.tile([128, CH], f32, name="psq", tag="proj")
                for d in range(ND):
                    nc.tensor.matmul(
                        ps,
                        lhsT=r(wq_sb[d][:, 128 * t:128 * (t + 1)]),
                        rhs=r(xt[d]),
                        start=(d == 0),
                        stop=(d == ND - 1),
                    )
                q_t = pq.tile([128, CH], f32r, name=f"q{t}", tag=f"q{t}")
                nc.vector.tensor_copy(out=q_t, in_=ps)
                qt.append(q_t)
            for t in range(NMT):
                ps = pp_proj.tile([128, CH], f32, name="psk", tag="proj")
                for d in range(ND):
                    nc.tensor.matmul(
                        ps,
                        lhsT=r(wk_sb[d][:, 128 * t:128 * (t + 1)]),
                        rhs=r(xt[d]),
                        start=(d == 0),
                        stop=(d == ND - 1),
                    )
                k_t = pkv.tile([128, CH], f32r, name=f"k{t}_{j}", tag=f"k{t}_{j}")
                nc.vector.tensor_copy(out=k_t, in_=ps)
                kt_sb[t][j] = k_t
            # V projection -> natural [s, m], strided into vau (65-col groups)
            for st in range(CH // 128):
                ps = pp_proj.tile([128, M], f32, name="psv", tag="proj")
                for d in range(ND):
                    nc.tensor.matmul(
                        ps,
                        lhsT=r(xt[d][:, 128 * st:128 * (st + 1)]),
                        rhs=r(wv_sb[d]),
                        start=(d == 0),
                        stop=(d == ND - 1),
                    )
                g = vau[4 * j + st]
                nc.vector.tensor_copy(
                    out=g.rearrange("p (h c) -> p h c", c=65)[:, :, 0:64],
                    in_=ps.rearrange("p (h c) -> p h c", c=64),
                )

            # ---- attention, one head-pair at a time ----
            ct = []
            for t in range(NMT):
                av = [
                    pp_av.tile([65, CH], f32, name=f"av{h}", tag="av")
                    for h in range(2)
                ]
                nkt = 4 * (j + 1)
                for kt in range(nkt):
                    dd = kt - 4 * j          # diagonal index (>=0 on diag)
                    qoff = 128 * dd if dd >= 0 else 0
                    n = CH - qoff
                    ck, ks = kt // 4, (kt % 4) * 128
                    for h in range(2):
                        pb = 64 * h
                        sc = pp_sc.tile([128, CH], f32, name="sc", tag="sc")
                        nc.tensor.matmul(
                            sc[:, 0:n],
                            lhsT=r(kt_sb[t][ck][pb:pb + 64, ks:ks + 128]),
                            rhs=r(qt[t][pb:pb + 64, qoff:CH]),
                            start=True,
                            stop=True,
                            tile_position=(pb, 0),
                        )
                        u = pu.tile([128, CH], f32r, name="u", tag="u")
                        nc.scalar.activation(
                            out=u[:, 0:n], in_=sc[:, 0:n], func=EXP, scale=0.125
                        )
                        if dd >= 0:
                            # keep where q_rel >= k_partition, else 0
                            nc.gpsimd.affine_select(
                                out=u[:, 0:n],
                                in_=u[:, 0:n],
                                compare_op=mybir.AluOpType.is_ge,
                                fill=0.0,
                                base=0,
                                channel_multiplier=-1,
                                pattern=[[1, n]],
                            )
                        ha = 2 * t + h
                        nc.tensor.matmul(
                            av[h][:, qoff:CH],
                            lhsT=r(vau[kt][:, 65 * ha:65 * ha + 65]),
                            rhs=r(u[:, 0:n]),
                            start=(kt == 0),
                            stop=(kt == nkt - 1),
                        )
                # normalize -> C^T [m, s]
                c_t = pct.tile([128, CH], f32r, name=f"c{t}", tag=f"c{t}")
                for h in range(2):
                    rec = psm.tile([1, CH], f32, name="rec", tag="rec")
                    nc.vector.reciprocal(out=rec, in_=av[h][64:65, :])
                    bc = psm.tile([64, CH], f32, name="bc", tag="bc")
                    nc.gpsimd.partition_broadcast(bc, rec)
                    nc.vector.tensor_mul(
                        c_t[64 * h:64 * (h + 1), :], av[h][0:64, :], bc
                    )
                ct.append(c_t)

            # ---- out projection (transposed): out^T[n, s] ----
            for nt in range(NNT):
                ps = pp_out.tile([128, CH], f32, name="pso", tag="out")
                for t in range(NMT):
                    nc.tensor.matmul(
                        ps,
                        lhsT=r(wo_sb[t][:, 128 * nt:128 * (nt + 1)]),
                        rhs=r(ct[t]),
                        start=(t == 0),
                        stop=(t == NMT - 1),
                    )
                o_sb = po.tile([128, CH], f32, name="osb", tag="o")
                nc.scalar.copy(out=o_sb, in_=ps)
                nc.sync.dma_start(
                    out=outT[128 * nt:128 * (nt + 1), CH * j:CH * (j + 1)],
                    in_=o_sb,
                )


_PROG = None


def _build():
    global _PROG
    if _PROG is not None:
        return _PROG
    import concourse.bacc as bacc
    import concourse.mybir as mybir
    import concourse.tile as tile

    f32 = mybir.dt.float32
    nc = bacc.Bacc(
        "TRN2", target_bir_lowering=False, debug=False, enable_asserts=False
    )
    xT = nc.dram_tensor("xT", [D, S], f32, kind="ExternalInput").ap()
    wq = nc.dram_tensor("wq", [D, M], f32, kind="ExternalInput").ap()
    wk = nc.dram_tensor("wk", [D, M], f32, kind="ExternalInput").ap()
    wv = nc.dram_tensor("wv", [D, M], f32, kind="ExternalInput").ap()
    wo = nc.dram_tensor("wo", [M, D], f32, kind="ExternalInput").ap()
    ones8 = nc.dram_tensor("ones8", [128, HPC], f32, kind="ExternalInput").ap()
    outT = nc.dram_tensor("outT", [D, S], f32, kind="ExternalOutput").ap()

    with tile.TileContext(nc) as tc:
        _emit(nc, tc, tile, mybir, (xT, wq, wk, wv, wo, ones8, outT))
    nc.compile()
    _PROG = nc
    return nc


def kernel(x, Wq, Wk, Wv, Wo, bo):
    global LAST_RESULT
    import os

    from concourse.bass_utils import run_bass_kernel_spmd

    x = np.asarray(x, dtype=np.float32)
    Wq = np.asarray(Wq, dtype=np.float32)
    Wk = np.asarray(Wk, dtype=np.float32)
    Wv = np.asarray(Wv, dtype=np.float32)
    Wo = np.asarray(Wo, dtype=np.float32)
    bo = np.asarray(bo, dtype=np.float32)

    nc = _build()

    in_maps = []
    for c in range(NCORE):
        b, g = c // 2, c % 2
        cols = slice(M * g, M * (g + 1))
        in_maps.append(
            {
                "xT": np.ascontiguousarray(x[b].T),
                "wq": np.ascontiguousarray(Wq[:, cols]),
                "wk": np.ascontiguousarray(Wk[:, cols]),
                "wv": np.ascontiguousarray(Wv[:, cols]),
                "wo": np.ascontiguousarray(Wo[cols, :]),
                "ones8": np.ones((128, HPC), dtype=np.float32),
            }
        )

    res = run_bass_kernel_spmd(
        nc,
        in_maps,
        list(range(NCORE)),
        trace=bool(os.environ.get("KERNEL_TRACE")),
        tmpdir=os.environ.get("KERNEL_TRACE_DIR") or None,
    )
    LAST_RESULT = res

    out = np.empty((B, S, D), dtype=np.float32)
    for b in range(B):
        acc = res.results[2 * b]["outT"] + res.results[2 * b + 1]["outT"]
        out[b] = acc.T + bo[None, :]
    return out


# revision 5
# speedup vs baseline: 1.2310x; 1.2310x over previous
"""Multi-head causal attention (B=4, S=2048, D=1024, H=16) on 8 TRN2 cores.

Sharding: core c -> batch c//2, head-group c%2 (8 heads, 512 of the 1024
QKV columns / Wo rows).  Each core runs a fused QKV->attention->out-proj
kernel on its shard; the host sums the two head-group partials per batch.

Per-core layout choices:
  - x is fed pre-transposed (xT [D, S]) so Q^T/K^T come out of the PE in
    [m, s] layout and V in natural [s, m] layout with no on-chip transposes.
  - scores are computed transposed (S^T [k, q]); softmax runs as
    exp (ScalarE, scale=1/8 fused) -> causal mask (gpsimd affine_select,
    fill=0, diagonal tiles only, masked q-ranges skipped entirely) ->
    attnV matmul with a ones-column appended to V (M=65) so the softmax
    denominator accumulates for free in PSUM row 64.
  - normalization: DVE reciprocal of row 64, gpsimd partition_broadcast,
    one DVE multiply into C^T [m, s].
  - out-proj emits out^T [n, s]; the host transposes back.
All matmul inputs are bf16 (1 cycle/row on the PE; fp32r is a 2-pass
format at ~2 cycles/row); accumulation stays fp32 in PSUM.
"""

import numpy as np

B, S, D = 4, 2048, 1024
H, DH = 16, 64
HPC = 8            # heads per core
M = HPC * DH       # 512: per-core qkv out dim / wo in dim
NCORE = 8
CH = 512           # q/s chunk size
NCH = S // CH      # 4
ND = D // 128      # 8  d-tiles (contraction for qkv proj)
NMT = M // 128     # 4  m-tiles (= head pairs)
NKT = S // 128     # 16 k-tiles
NNT = D // 128     # 8  n-tiles (out proj)

LAST_RESULT = None  # BassKernelResults of the most recent run (for test.py)


def _emit(nc, tc, tile, mybir, aps):
    import concourse.bass as bass  # noqa: F401

    f32 = mybir.dt.float32
    bf16 = mybir.dt.bfloat16
    EXP = mybir.ActivationFunctionType.Exp
    xT, wq, wk, wv, wo, ones8, outT = aps

    def r(ap):
        return ap

    with (
        tc.tile_pool(name="w", bufs=1) as pw,
        tc.tile_pool(name="kv", bufs=1) as pkv,
        tc.tile_pool(name="qt", bufs=2) as pq,
        tc.tile_pool(name="ct", bufs=1) as pct,
        tc.tile_pool(name="x", bufs=1) as px,
        tc.tile_pool(name="u", bufs=4) as pu,
        tc.tile_pool(name="sm", bufs=2) as psm,
        tc.tile_pool(name="o", bufs=2) as po,
        tc.tile_pool(name="ps_proj", bufs=2, space="PSUM") as pp_proj,
        tc.tile_pool(name="ps_sc", bufs=2, space="PSUM") as pp_sc,
        tc.tile_pool(name="ps_av", bufs=2, space="PSUM") as pp_av,
        tc.tile_pool(name="ps_out", bufs=2, space="PSUM") as pp_out,
    ):
        # ---- weights ----
        wq_sb, wk_sb, wv_sb = [], [], []
        for d in range(ND):
            for lst, src, nm in ((wq_sb, wq, "wq"), (wk_sb, wk, "wk"), (wv_sb, wv, "wv")):
                t = pw.tile([128, M], bf16, name=f"{nm}{d}", tag=f"{nm}{d}")
                nc.sync.dma_start(out=t, in_=src[128 * d:128 * (d + 1), :])
                lst.append(t)
        wo_sb = []
        for t in range(NMT):
            w = pw.tile([128, D], bf16, name=f"wo{t}", tag=f"wo{t}")
            nc.sync.dma_start(out=w, in_=wo[128 * t:128 * (t + 1), :])
            wo_sb.append(w)

        # ---- V storage: [s, 8 heads x (64 V + 1 ones)] ----
        vau = []
        for st in range(NKT):
            v = pkv.tile([128, HPC * 65], bf16, name=f"vau{st}", tag=f"vau{st}")
            nc.sync.dma_start(
                out=v.rearrange("p (h c) -> p h c", c=65)[:, :, 64:65],
                in_=ones8.rearrange("p (h c) -> p h c", c=1),
            )
            vau.append(v)
        kt_sb = [[None] * NCH for _ in range(NMT)]

        for j in range(NCH):  # ---- chunk loop ----
            # x^T chunk [d, s]
            xt = []
            for d in range(ND):
                x_t = px.tile([128, CH], bf16, name=f"x{d}", tag=f"x{d}")
                nc.sync.dma_start(
                    out=x_t, in_=xT[128 * d:128 * (d + 1), CH * j:CH * (j + 1)]
                )
                xt.append(x_t)

            # Q^T, K^T projections -> [m, s]
            qt = []
            for t in range(NMT):
                ps = pp_proj.tile([128, CH], f32, name="psq", tag="proj")
                for d in range(ND):
                    nc.tensor.matmul(
                        ps,
                        lhsT=r(wq_sb[d][:, 128 * t:128 * (t + 1)]),
                        rhs=r(xt[d]),
                        start=(d == 0),
                        stop=(d == ND - 1),
                    )
                q_t = pq.tile([128, CH], bf16, name=f"q{t}", tag=f"q{t}")
                nc.vector.tensor_copy(out=q_t, in_=ps)
                qt.append(q_t)
            for t in range(NMT):
                ps = pp_proj.tile([128, CH], f32, name="psk", tag="proj")
                for d in range(ND):
                    nc.tensor.matmul(
                        ps,
                        lhsT=r(wk_sb[d][:, 128 * t:128 * (t + 1)]),
                        rhs=r(xt[d]),
                        start=(d == 0),
                        stop=(d == ND - 1),
                    )
                k_t = pkv.tile([128, CH], bf16, name=f"k{t}_{j}", tag=f"k{t}_{j}")
                nc.vector.tensor_copy(out=k_t, in_=ps)
                kt_sb[t][j] = k_t
            # V projection -> natural [s, m], strided into vau (65-col groups)
            for st in range(CH // 128):
                ps = pp_proj.tile([128, M], f32, name="psv", tag="proj")
                for d in range(ND):
                    nc.tensor.matmul(
                        ps,
                        lhsT=r(xt[d][:, 128 * st:128 * (st + 1)]),
                        rhs=r(wv_sb[d]),
                        start=(d == 0),
                        stop=(d == ND - 1),
                    )
                g = vau[4 * j + st]
                nc.vector.tensor_copy(
                    out=g.rearrange("p (h c) -> p h c", c=65)[:, :, 0:64],
                    in_=ps.rearrange("p (h c) -> p h c", c=64),
                )

            # ---- attention, one head-pair at a time ----
            ct = []
            for t in range(NMT):
                av = [
                    pp_av.tile([65, CH], f32, name=f"av{h}", tag="av")
                    for h in range(2)
                ]
                nkt = 4 * (j + 1)
                for kt in range(nkt):
                    dd = kt - 4 * j          # diagonal index (>=0 on diag)
                    qoff = 128 * dd if dd >= 0 else 0
                    n = CH - qoff
                    ck, ks = kt // 4, (kt % 4) * 128
                    for h in range(2):
                        pb = 64 * h
                        sc = pp_sc.tile([128, CH], f32, name="sc", tag="sc")
                        nc.tensor.matmul(
                            sc[:, 0:n],
                            lhsT=r(kt_sb[t][ck][pb:pb + 64, ks:ks + 128]),
                            rhs=r(qt[t][pb:pb + 64, qoff:CH]),
                            start=True,
                            stop=True,
                            tile_position=(pb, 0),
                        )
                        u = pu.tile([128, CH], bf16, name="u", tag="u")
                        nc.scalar.activation(
                            out=u[:, 0:n], in_=sc[:, 0:n], func=EXP, scale=0.125
                        )
                        if dd >= 0:
                            # keep where q_rel >= k_partition, else 0
                            nc.gpsimd.affine_select(
                                out=u[:, 0:n],
                                in_=u[:, 0:n],
                                compare_op=mybir.AluOpType.is_ge,
                                fill=0.0,
                                base=0,
                                channel_multiplier=-1,
                                pattern=[[1, n]],
                            )
                        ha = 2 * t + h
                        nc.tensor.matmul(
                            av[h][:, qoff:CH],
                            lhsT=r(vau[kt][:, 65 * ha:65 * ha + 65]),
                            rhs=r(u[:, 0:n]),
                            start=(kt == 0),
                            stop=(kt == nkt - 1),
                        )
                # normalize -> C^T [m, s]
                c_t = pct.tile([128, CH], bf16, name=f"c{t}", tag=f"c{t}")
                for h in range(2):
                    rec = psm.tile([1, CH], f32, name="rec", tag="rec")
                    nc.vector.reciprocal(out=rec, in_=av[h][64:65, :])
                    bc = psm.tile([64, CH], f32, name="bc", tag="bc")
                    nc.gpsimd.partition_broadcast(bc, rec)
                    nc.vector.tensor_mul(
                        c_t[64 * h:64 * (h + 1), :], av[h][0:64, :], bc
                    )
                ct.append(c_t)

            # ---- out projection (transposed): out^T[n, s] ----
            for nt in range(NNT):
                ps = pp_out.tile([128, CH], f32, name="pso", tag="out")
                for t in range(NMT):
                    nc.tensor.matmul(
                        ps,
                        lhsT=r(wo_sb[t][:, 128 * nt:128 * (nt + 1)]),
                        rhs=r(ct[t]),
                        start=(t == 0),
                        stop=(t == NMT - 1),
                    )
                o_sb = po.tile([128, CH], f32, name="osb", tag="o")
                nc.scalar.copy(out=o_sb, in_=ps)
                nc.sync.dma_start(
                    out=outT[128 * nt:128 * (nt + 1), CH * j:CH * (j + 1)],
                    in_=o_sb,
                )


_PROG = None


def _build():
    global _PROG
    if _PROG is not None:
        return _PROG
    import concourse.bacc as bacc
    import concourse.mybir as mybir
    import concourse.tile as tile

    f32 = mybir.dt.float32
    bf16 = mybir.dt.bfloat16
    nc = bacc.Bacc(
        "TRN2", target_bir_lowering=False, debug=False, enable_asserts=False
    )
    xT = nc.dram_tensor("xT", [D, S], bf16, kind="ExternalInput").ap()
    wq = nc.dram_tensor("wq", [D, M], bf16, kind="ExternalInput").ap()
    wk = nc.dram_tensor("wk", [D, M], bf16, kind="ExternalInput").ap()
    wv = nc.dram_tensor("wv", [D, M], bf16, kind="ExternalInput").ap()
    wo = nc.dram_tensor("wo", [M, D], bf16, kind="ExternalInput").ap()
    ones8 = nc.dram_tensor("ones8", [128, HPC], bf16, kind="ExternalInput").ap()
    outT = nc.dram_tensor("outT", [D, S], f32, kind="ExternalOutput").ap()

    with tile.TileContext(nc) as tc:
        _emit(nc, tc, tile, mybir, (xT, wq, wk, wv, wo, ones8, outT))
    nc.compile()
    _PROG = nc
    return nc


def kernel(x, Wq, Wk, Wv, Wo, bo):
    global LAST_RESULT
    import os

    from concourse.bass_utils import run_bass_kernel_spmd

    x = np.asarray(x, dtype=np.float32)
    Wq = np.asarray(Wq, dtype=np.float32)
    Wk = np.asarray(Wk, dtype=np.float32)
    Wv = np.asarray(Wv, dtype=np.float32)
    Wo = np.asarray(Wo, dtype=np.float32)
    bo = np.asarray(bo, dtype=np.float32)

    nc = _build()

    import ml_dtypes

    bf = ml_dtypes.bfloat16
    in_maps = []
    for c in range(NCORE):
        b, g = c // 2, c % 2
        cols = slice(M * g, M * (g + 1))
        in_maps.append(
            {
                "xT": np.ascontiguousarray(x[b].T).astype(bf),
                "wq": np.ascontiguousarray(Wq[:, cols]).astype(bf),
                "wk": np.ascontiguousarray(Wk[:, cols]).astype(bf),
                "wv": np.ascontiguousarray(Wv[:, cols]).astype(bf),
                "wo": np.ascontiguousarray(Wo[cols, :]).astype(bf),
                "ones8": np.ones((128, HPC), dtype=bf),
            }
        )

    res = run_bass_kernel_spmd(
        nc,
        in_maps,
        list(range(NCORE)),
        trace=bool(os.environ.get("KERNEL_TRACE")),
        tmpdir=os.environ.get("KERNEL_TRACE_DIR") or None,
    )
    LAST_RESULT = res

    out = np.empty((B, S, D), dtype=np.float32)
    for b in range(B):
        acc = res.results[2 * b]["outT"] + res.results[2 * b + 1]["outT"]
        out[b] = acc.T + bo[None, :]
    return out


# revision 9
# speedup vs baseline: 1.2872x; 1.0456x over previous
"""Multi-head causal attention (B=4, S=2048, D=1024, H=16) on 8 TRN2 cores.

Sharding: core c -> batch c//2, head-group c%2 (8 heads, 512 of the 1024
QKV columns / Wo rows).  Each core runs a fused QKV->attention->out-proj
kernel on its shard; the host sums the two head-group partials per batch.

Per-core layout choices:
  - x is fed pre-transposed (xT [D, S]) so Q^T/K^T come out of the PE in
    [m, s] layout and V in natural [s, m] layout with no on-chip transposes.
  - scores are computed transposed (S^T [k, q]); softmax runs as
    exp (ScalarE, scale=1/8 fused) -> causal mask (gpsimd affine_select,
    fill=0, diagonal tiles only, masked q-ranges skipped entirely) ->
    attnV matmul with a ones-column appended to V (M=65) so the softmax
    denominator accumulates for free in PSUM row 64.
  - normalization: DVE reciprocal of row 64, gpsimd partition_broadcast,
    one DVE multiply into C^T [m, s].
  - out-proj emits out^T [n, s]; the host transposes back.
All matmul inputs are bf16 (1 cycle/row on the PE; fp32r is a 2-pass
format at ~2 cycles/row); accumulation stays fp32 in PSUM.
"""

import numpy as np

B, S, D = 4, 2048, 1024
H, DH = 16, 64
HPC = 8            # heads per core
M = HPC * DH       # 512: per-core qkv out dim / wo in dim
NCORE = 8
CH = 512           # q/s chunk size
NCH = S // CH      # 4
ND = D // 128      # 8  d-tiles (contraction for qkv proj)
NMT = M // 128     # 4  m-tiles (= head pairs)
NKT = S // 128     # 16 k-tiles
NNT = D // 128     # 8  n-tiles (out proj)

LAST_RESULT = None  # BassKernelResults of the most recent run (for test.py)


def _emit(nc, tc, tile, mybir, aps):
    import concourse.bass as bass  # noqa: F401

    f32 = mybir.dt.float32
    bf16 = mybir.dt.bfloat16
    EXP = mybir.ActivationFunctionType.Exp
    xT, wq, wk, wv, wo, ones8, outT = aps

    def r(ap):
        return ap

    with (
        tc.tile_pool(name="w", bufs=1) as pw,
        tc.tile_pool(name="kv", bufs=1) as pkv,
        tc.tile_pool(name="qt", bufs=2) as pq,
        tc.tile_pool(name="ct", bufs=1) as pct,
        tc.tile_pool(name="x", bufs=1) as px,
        tc.tile_pool(name="u", bufs=4) as pu,
        tc.tile_pool(name="sm", bufs=2) as psm,
        tc.tile_pool(name="o", bufs=2) as po,
        tc.tile_pool(name="ps_mm", bufs=2, space="PSUM") as pp_mm,
        tc.tile_pool(name="ps_sc", bufs=2, space="PSUM") as pp_sc,
        tc.tile_pool(name="ps_av", bufs=2, space="PSUM") as pp_av,
    ):
        # ---- weights ----
        wq_sb, wk_sb, wv_sb = [], [], []
        for d in range(ND):
            for lst, src, nm in ((wq_sb, wq, "wq"), (wk_sb, wk, "wk"), (wv_sb, wv, "wv")):
                t = pw.tile([128, M], bf16, name=f"{nm}{d}", tag=f"{nm}{d}")
                nc.sync.dma_start(out=t, in_=src[128 * d:128 * (d + 1), :])
                lst.append(t)
        wo_sb = []
        for t in range(NMT):
            w = pw.tile([128, D], bf16, name=f"wo{t}", tag=f"wo{t}")
            nc.sync.dma_start(out=w, in_=wo[128 * t:128 * (t + 1), :])
            wo_sb.append(w)

        # ---- V storage: [s, 8 heads x (64 V + 1 ones)] ----
        vau = []
        for st in range(NKT):
            v = pkv.tile([128, HPC * 65], bf16, name=f"vau{st}", tag=f"vau{st}")
            nc.sync.dma_start(
                out=v.rearrange("p (h c) -> p h c", c=65)[:, :, 64:65],
                in_=ones8.rearrange("p (h c) -> p h c", c=1),
            )
            vau.append(v)
        kt_sb = [[None] * NCH for _ in range(NMT)]

        for j in range(NCH):  # ---- chunk loop ----
            # x^T chunk [d, s]
            xt = []
            for d in range(ND):
                x_t = px.tile([128, CH], bf16, name=f"x{d}", tag=f"x{d}")
                nc.sync.dma_start(
                    out=x_t, in_=xT[128 * d:128 * (d + 1), CH * j:CH * (j + 1)]
                )
                xt.append(x_t)

            # Q^T, K^T projections -> [m, s]
            qt = []
            for t in range(NMT):
                ps = pp_mm.tile([128, CH], f32, name="psq", tag="mm")
                for d in range(ND):
                    nc.tensor.matmul(
                        ps,
                        lhsT=r(wq_sb[d][:, 128 * t:128 * (t + 1)]),
                        rhs=r(xt[d]),
                        start=(d == 0),
                        stop=(d == ND - 1),
                    )
                q_t = pq.tile([128, CH], bf16, name=f"q{t}", tag=f"q{t}")
                nc.vector.tensor_copy(out=q_t, in_=ps)
                qt.append(q_t)
            for t in range(NMT):
                ps = pp_mm.tile([128, CH], f32, name="psk", tag="mm")
                for d in range(ND):
                    nc.tensor.matmul(
                        ps,
                        lhsT=r(wk_sb[d][:, 128 * t:128 * (t + 1)]),
                        rhs=r(xt[d]),
                        start=(d == 0),
                        stop=(d == ND - 1),
                    )
                k_t = pkv.tile([128, CH], bf16, name=f"k{t}_{j}", tag=f"k{t}_{j}")
                nc.vector.tensor_copy(out=k_t, in_=ps)
                kt_sb[t][j] = k_t
            # V projection -> natural [s, m], strided into vau (65-col groups)
            for st in range(CH // 128):
                ps = pp_mm.tile([128, M], f32, name="psv", tag="mm")
                for d in range(ND):
                    nc.tensor.matmul(
                        ps,
                        lhsT=r(xt[d][:, 128 * st:128 * (st + 1)]),
                        rhs=r(wv_sb[d]),
                        start=(d == 0),
                        stop=(d == ND - 1),
                    )
                g = vau[4 * j + st]
                nc.vector.tensor_copy(
                    out=g.rearrange("p (h c) -> p h c", c=65)[:, :, 0:64],
                    in_=ps.rearrange("p (h c) -> p h c", c=64),
                )

            # ---- attention, one head-pair at a time ----
            ct = []
            for t in range(NMT):
                av = [
                    pp_av.tile([65, CH], f32, name=f"av{h}", tag="av")
                    for h in range(2)
                ]
                nkt = 4 * (j + 1)
                for kt in range(nkt):
                    dd = kt - 4 * j          # diagonal index (>=0 on diag)
                    qoff = 128 * dd if dd >= 0 else 0
                    n = CH - qoff
                    ck, ks = kt // 4, (kt % 4) * 128
                    # both heads' scores in one 2-bank PSUM tile
                    sc = pp_sc.tile([128, 2 * CH], f32, name="sc", tag="sc")
                    for h in range(2):
                        pb = 64 * h
                        nc.tensor.matmul(
                            sc[:, CH * h:CH * h + n],
                            lhsT=r(kt_sb[t][ck][pb:pb + 64, ks:ks + 128]),
                            rhs=r(qt[t][pb:pb + 64, qoff:CH]),
                            start=True,
                            stop=True,
                            tile_position=(pb, 0),
                        )
                    u = pu.tile([128, 2 * CH], bf16, name="u", tag="u")
                    scv = sc.rearrange("p (h q) -> p h q", h=2)[:, :, 0:n]
                    uv = u.rearrange("p (h q) -> p h q", h=2)[:, :, 0:n]
                    nc.scalar.activation(out=uv, in_=scv, func=EXP, scale=0.125)
                    if dd >= 0:
                        # keep where q_rel >= k_partition (same mask both heads)
                        nc.gpsimd.affine_select(
                            out=uv,
                            in_=uv,
                            compare_op=mybir.AluOpType.is_ge,
                            fill=0.0,
                            base=0,
                            channel_multiplier=-1,
                            pattern=[[0, 2], [1, n]],
                        )
                    for h in range(2):
                        ha = 2 * t + h
                        nc.tensor.matmul(
                            av[h][:, qoff:CH],
                            lhsT=r(vau[kt][:, 65 * ha:65 * ha + 65]),
                            rhs=r(u[:, CH * h:CH * h + n]),
                            start=(kt == 0),
                            stop=(kt == nkt - 1),
                        )
                # normalize -> C^T [m, s]
                c_t = pct.tile([128, CH], bf16, name=f"c{t}", tag=f"c{t}")
                for h in range(2):
                    rec = psm.tile([1, CH], f32, name="rec", tag="rec")
                    nc.vector.reciprocal(out=rec, in_=av[h][64:65, :])
                    bc = psm.tile([64, CH], f32, name="bc", tag="bc")
                    nc.gpsimd.partition_broadcast(bc, rec)
                    nc.vector.tensor_mul(
                        c_t[64 * h:64 * (h + 1), :], av[h][0:64, :], bc
                    )
                ct.append(c_t)

            # ---- out projection (transposed): out^T[n, s] ----
            for nt in range(NNT):
                ps = pp_mm.tile([128, CH], f32, name="pso", tag="mm")
                for t in range(NMT):
                    nc.tensor.matmul(
                        ps,
                        lhsT=r(wo_sb[t][:, 128 * nt:128 * (nt + 1)]),
                        rhs=r(ct[t]),
                        start=(t == 0),
                        stop=(t == NMT - 1),
                    )
                o_sb = po.tile([128, CH], f32, name="osb", tag="o")
                nc.vector.tensor_copy(out=o_sb, in_=ps)
                nc.sync.dma_start(
                    out=outT[128 * nt:128 * (nt + 1), CH * j:CH * (j + 1)],
                    in_=o_sb,
                )


_PROG = None


def _build():
    global _PROG
    if _PROG is not None:
        return _PROG
    import concourse.bacc as bacc
    import concourse.mybir as mybir
    import concourse.tile as tile

    f32 = mybir.dt.float32
    bf16 = mybir.dt.bfloat16
    nc = bacc.Bacc(
        "TRN2", target_bir_lowering=False, debug=False, enable_asserts=False
    )
    xT = nc.dram_tensor("xT", [D, S], bf16, kind="ExternalInput").ap()
    wq = nc.dram_tensor("wq", [D, M], bf16, kind="ExternalInput").ap()
    wk = nc.dram_tensor("wk", [D, M], bf16, kind="ExternalInput").ap()
    wv = nc.dram_tensor("wv", [D, M], bf16, kind="ExternalInput").ap()
    wo = nc.dram_tensor("wo", [M, D], bf16, kind="ExternalInput").ap()
    ones8 = nc.dram_tensor("ones8", [128, HPC], bf16, kind="ExternalInput").ap()
    outT = nc.dram_tensor("outT", [D, S], f32, kind="ExternalOutput").ap()

    with tile.TileContext(nc) as tc:
        _emit(nc, tc, tile, mybir, (xT, wq, wk, wv, wo, ones8, outT))
    nc.compile()
    _PROG = nc
    return nc


def kernel(x, Wq, Wk, Wv, Wo, bo):
    global LAST_RESULT
    import os

    from concourse.bass_utils import run_bass_kernel_spmd

    x = np.asarray(x, dtype=np.float32)
    Wq = np.asarray(Wq, dtype=np.float32)
    Wk = np.asarray(Wk, dtype=np.float32)
    Wv = np.asarray(Wv, dtype=np.float32)
    Wo = np.asarray(Wo, dtype=np.float32)
    bo = np.asarray(bo, dtype=np.float32)

    nc = _build()

    import ml_dtypes

    bf = ml_dtypes.bfloat16
    in_maps = []
    for c in range(NCORE):
        b, g = c // 2, c % 2
        cols = slice(M * g, M * (g + 1))
        in_maps.append(
            {
                "xT": np.ascontiguousarray(x[b].T).astype(bf),
                "wq": np.ascontiguousarray(Wq[:, cols]).astype(bf),
                "wk": np.ascontiguousarray(Wk[:, cols]).astype(bf),
                "wv": np.ascontiguousarray(Wv[:, cols]).astype(bf),
                "wo": np.ascontiguousarray(Wo[cols, :]).astype(bf),
                "ones8": np.ones((128, HPC), dtype=bf),
            }
        )

    res = run_bass_kernel_spmd(
        nc,
        in_maps,
        list(range(NCORE)),
        trace=bool(os.environ.get("KERNEL_TRACE")),
        tmpdir=os.environ.get("KERNEL_TRACE_DIR") or None,
    )
    LAST_RESULT = res

    out = np.empty((B, S, D), dtype=np.float32)
    for b in range(B):
        acc = res.results[2 * b]["outT"] + res.results[2 * b + 1]["outT"]
        out[b] = acc.T + bo[None, :]
    return out
